# revision 24
# baseline (speedup 1.0000x reference)
"""Trainium2 Bass kernel for nn_Decoder_40338332844507.

Computes logits = einsum('btc,wpc->bptw', q, W) + b.T[None,:,None,:]
with q [32, 2048, 256] f32, W [49, 32, 256] f32, b [49, 32] f32,
output [32, 32, 2048, 49] f32.

Strategy: data-parallel over batch across 8 NeuronCores (4 batches per
core). Per core, for each 128-token tile the TensorEngine computes
out[t, (p,w)] = qT_tile.T @ Wr in bf16. The device stores the logits
in bf16 (halves the dominant HBM store stream vs f32; rel err stays
~5e-3, well under the 2e-2 gate); the host upcasts to f32 and fuses
the bias add into the upcast. PSUM->SBUF eviction is a pure copy
alternating between the Vector (DVE) and Scalar (Act) engines per
token-tile so neither engine gates the TensorEngine. Token tiles are
strided (t = tp*TL + tl, partition dim = tp) so each output store
covers contiguous DRAM runs of 16*49*2 bytes. The first batch is
computed in p-halves so stores start early; the last batch ends in
p-quarters so the final un-overlapped store tail is only ~1.6 MB.
"""

import json
import sys
import numpy as np
from contextlib import ExitStack

if "/opt/trn_rl_repo" not in sys.path:
    sys.path.insert(0, "/opt/trn_rl_repo")

import concourse.bass as bass
import concourse.tile as tile
from concourse import mybir
from concourse.bass_utils import run_bass_kernel_spmd

B, T, C = 32, 2048, 256
P, WW = 32, 49
N = P * WW  # 1568
N_CORES = 8
B_LOC = B // N_CORES  # 4 batches per core
TL = 16  # token interleave: t = tp*16 + tl -> store runs of 16*49*2 B


def _patch_split_sync_waits():
    """The walrus build on this image accepts at most ONE sync-wait per
    instruction ("Too many sync wait commands" otherwise). Tile emits
    instructions with several waits. Post-process the serialized BIR:
    hoist all but the last wait of each instruction onto 1-wait NoOps
    inserted immediately before it on the same engine (engines execute
    their instruction stream in order, so the semantics are identical)."""
    if getattr(bass.Bass, "_split_waits_patched", False):
        return
    orig = bass.Bass.to_json_bytes

    def to_json_bytes(self):
        m = json.loads(orig(self))
        ctr = 0
        for f in m.get("functions", []):
            for bb in f.get("blocks", []):
                out = []
                for inst in bb.get("instructions", []):
                    si = inst.get("sync_info")
                    if si:
                        waits = si.get("on_wait") or []
                        if len(waits) > 1:
                            for wt in waits[:-1]:
                                ctr += 1
                                nop = {
                                    "engine": inst["engine"],
                                    "ins": [],
                                    "outs": [],
                                    "name": f"I-npw{ctr}",
                                    "opcode": "NoOp",
                                    "sync_info": {"on_wait": [wt], "on_update": []},
                                }
                                if inst.get("debug") is not None:
                                    nop["debug"] = inst["debug"]
                                out.append(nop)
                            si["on_wait"] = waits[-1:]
                    out.append(inst)
                bb["instructions"] = out
        return json.dumps(m).encode()

    bass.Bass.to_json_bytes = to_json_bytes
    bass.Bass._split_waits_patched = True


def build_bass():
    _patch_split_sync_waits()
    nc = bass.Bass("TRN2", target_bir_lowering=False, debug=False)
    f32 = mybir.dt.float32
    bf16 = mybir.dt.bfloat16

    qt = nc.dram_tensor("qt", [B_LOC, C, T], bf16, kind="ExternalInput")
    wr = nc.dram_tensor("wr", [C, N], bf16, kind="ExternalInput")
    o = nc.dram_tensor("o", [B_LOC, P, T, WW], bf16, kind="ExternalOutput")

    with tile.TileContext(nc) as tc:
        with ExitStack() as ctx:
            consts = ctx.enter_context(tc.tile_pool(name="consts", bufs=1))
            qpool = ctx.enter_context(tc.tile_pool(name="qpool", bufs=3))
            opool = ctx.enter_context(tc.tile_pool(name="opool", bufs=3))
            psum = ctx.enter_context(tc.tile_pool(name="psum", bufs=2, space="PSUM"))

            wr_sb = [
                consts.tile([128, N], bf16, tag=f"wr{k}", name=f"wr{k}")
                for k in range(2)
            ]

            # PE warm-up: the HAM clock gate keeps the PE at 1.2 GHz until
            # it has seen ~3.4us of sustained activity. Chew on dummy
            # matmuls during the initial load window so the real matmuls
            # start at 2.4 GHz.
            dummy = consts.tile([128, 128], bf16, tag="dummy", name="dummy")
            nc.gpsimd.memset(dummy[:], 0.0)
            pt_warm = psum.tile([128, 1024], f32, tag="pta", name="pt_warm")
            for i in range(48):
                nc.tensor.matmul(
                    pt_warm[:, 0:128], dummy[:], dummy[:], start=True, stop=True
                )

            state = {"st": 0}
            # per-tl eviction split DVE/Act. Tile's dependency tracking
            # serializes any two engines that touch the SAME tile (even
            # disjoint ranges), so the DVE and Act eviction paths must not
            # share anything: separate PSUM tiles (matmuls write both) and
            # separate output sbuf tiles. All PSUM tiles are <=1 bank
            # (np_<=16) so bufs=4 fits in the 8 banks and the 4-deep
            # pipeline absorbs cross-engine semaphore latency.
            # DVE 1x = elems/0.96 ns, Act = (elems+352)/1.2 ns. Equal p
            # splits keep the two store queues byte-balanced so they drain
            # concurrently at the end (both evictions still fit under the
            # PE's per-tl time).
            EV_DVE = {32: 16, 16: 8, 12: 6, 8: 4, 4: 4}

            def unit(b, q_v, p0, np_, uname):
                """Compute o[b, p0:p0+np_, :, :] (all tokens) and store it."""
                pd = EV_DVE[np_]
                pa = np_ - pd  # Act's share (may be 0)
                wa, wb = pd * WW, pa * WW
                oha = opool.tile(
                    [128, pd, TL * WW], bf16, tag="oha", name=f"oha_{uname}"
                )
                ohb = (
                    opool.tile(
                        [128, pa, TL * WW], bf16, tag="ohb", name=f"ohb_{uname}"
                    )
                    if pa
                    else None
                )
                for tl in range(TL):
                    pta = psum.tile([128, 1024], f32, tag="pta", name=f"pa_{uname}_{tl}")
                    ptb = (
                        psum.tile([128, 1024], f32, tag="ptb", name=f"pb_{uname}_{tl}")
                        if pa
                        else None
                    )
                    for k in range(2):
                        for pt, base, width in ((pta, p0 * WW, wa), (ptb, p0 * WW + wa, wb)):
                            if not width:
                                continue
                            for n0 in range(0, width, 512):
                                n1 = min(n0 + 512, width)
                                nc.tensor.matmul(
                                    pt[:, n0:n1],
                                    q_v[k][:, tl, :],
                                    wr_sb[k][:, base + n0 : base + n1],
                                    start=(k == 0),
                                    stop=(k == 1),
                                )
                    ds_ = bass.ds(tl * WW, WW)
                    pva = pta[:, :wa].rearrange("t (p w) -> t p w", w=WW)
                    nc.vector.tensor_copy(oha[:, :, ds_], pva[:])
                    if pa:
                        pvb = ptb[:, :wb].rearrange("t (p w) -> t p w", w=WW)
                        nc.scalar.copy(ohb[:, :, ds_], pvb[:])
                # store each engine's tile. oha (DVE path) always goes via
                # sync; ohb (Act path) always via scalar, whose dma_start
                # wait is then satisfied by program order, so a store issue
                # can never block the other engine's eviction stream.
                for tile_, eng, ps, pe_ in (
                    (oha, nc.sync, p0, p0 + pd),
                    (ohb, nc.scalar, p0 + pd, p0 + np_),
                ):
                    if tile_ is None or pe_ == ps:
                        continue
                    d = (
                        o.ap()[b, ps:pe_, :, :]
                        .rearrange("p (t l) w -> t p (l w)", l=TL)
                    )
                    eng.dma_start(d, tile_[:, :, :])

            # q[b] transposed: two [128(c), 2048(t)] bf16 tiles per batch.
            # t split as (tp, tl); lhsT tiles are [c, tp] (stride TL).
            qs = {}

            def load_q(b, eng0, eng1):
                q_sb = [
                    qpool.tile([128, T], bf16, tag=f"q{k}", name=f"q{k}_{b}")
                    for k in range(2)
                ]
                eng0.dma_start(q_sb[0][:], qt.ap()[b, 0:128, :])
                eng1.dma_start(q_sb[1][:], qt.ap()[b, 128:256, :])
                qs[b] = [
                    q_sb[k][:].rearrange("c (p l) -> c l p", l=TL) for k in range(2)
                ]

            # Early loads ride the two HWDGE rings, which fair-share HBM:
            # tl0-k0 needs q00+wr0 -> first on each ring; k1's pair second;
            # then b3's and b1's q (needed at ~26us / ~49us). b2 goes via
            # gpsimd/SWDGE, gated by the q-buffer reuse semaphores.
            q_sb0 = [
                qpool.tile([128, T], bf16, tag=f"q{k}", name=f"q{k}_0")
                for k in range(2)
            ]
            nc.sync.dma_start(q_sb0[0][:], qt.ap()[0, 0:128, :])
            nc.scalar.dma_start(wr_sb[0][:], wr.ap()[0:128, :])
            nc.sync.dma_start(wr_sb[1][:], wr.ap()[128:256, :])
            nc.scalar.dma_start(q_sb0[1][:], qt.ap()[0, 128:256, :])
            qs[0] = [
                q_sb0[k][:].rearrange("c (p l) -> c l p", l=TL) for k in range(2)
            ]
            load_q(3, nc.sync, nc.scalar)
            load_q(1, nc.sync, nc.scalar)

            # Unit order: fulls (b0, b1) amortize per-matmul overhead best;
            # b3h0 is interleaved early and the batch tail tapers
            # (16p halves -> 12p -> 4p) so the store stream stays fed and
            # the final un-overlapped store is only ~0.8 MB.
            unit(0, qs[0], 0, 32, "b0")
            load_q(2, nc.gpsimd, nc.gpsimd)
            unit(3, qs[3], 0, 16, "b3h0")
            unit(1, qs[1], 0, 32, "b1")
            unit(2, qs[2], 0, 16, "b2h0")
            unit(2, qs[2], 16, 16, "b2h1")
            unit(3, qs[3], 16, 12, "b3t12")
            unit(3, qs[3], 28, 4, "b3e7")
    return nc


_NC_CACHE = None


def _get_nc():
    global _NC_CACHE
    if _NC_CACHE is None:
        _NC_CACHE = build_bass()
    return _NC_CACHE


def prep_core_inputs(q, W):
    """Host-side layout prep: activation transpose + weight packing, bf16."""
    import ml_dtypes

    bf = ml_dtypes.bfloat16
    q = np.asarray(q, dtype=np.float32)
    Wt = np.asarray(W, dtype=np.float32)
    qt = np.ascontiguousarray(q.transpose(0, 2, 1).astype(bf))  # [B, C, T]
    wr = np.ascontiguousarray(Wt.transpose(2, 1, 0).reshape(C, N).astype(bf))
    return [
        {"qt": qt[c * B_LOC : (c + 1) * B_LOC], "wr": wr}
        for c in range(N_CORES)
    ]


def assemble_output(res, bvec):
    """Gather per-core bf16 logits, upcast to f32 and fuse the bias add."""
    bias = np.asarray(bvec, dtype=np.float32).T[None, :, None, :]  # [1,P,1,W]
    out = np.empty((B, P, T, WW), dtype=np.float32)
    for c in range(N_CORES):
        sl = slice(c * B_LOC, (c + 1) * B_LOC)
        out[sl] = res.results[c]["o"].astype(np.float32)
        out[sl] += bias
    return out


def kernel(q, W, b):
    nc = _get_nc()
    in_maps = prep_core_inputs(q, W)
    res = run_bass_kernel_spmd(nc, in_maps, core_ids=list(range(N_CORES)))
    return assemble_output(res, b)
